# revision 4
# baseline (speedup 1.0000x reference)
"""BinaryLinear kernel for 8x TRN2 NeuronCores.

out = x @ (weight > 0)  with x [8192, 2048] f32, weight [2048, 2048] f32.

Sharding: data-parallel over batch (1024 rows/core), weight replicated.

Per core (M=1024, K=2048, N=2048). v3 schedule (from TimelineSim
engine-occupancy iteration; v1 168us sim -> v2 140us -> v3):

- Weight streams in three column tranches (1024 + 512 + 512 cols),
  k-tile-major within each, so all 16 k-tiles of the first tranche are
  resident early; later tranches arrive with slack while earlier
  matmul groups execute. Binarize to exact bf16 {0,1} on DVE.
- DMA order on the SP ring interleaves the startup critically:
  x0h0, w-kt0, x0h1, w-kt1, x1, w-kt2, x2, w-kt3..15, x3..x7,
  tranche2 k-tiles, tranche3 k-tiles. The first matmul issues ~5us in.
- Ramp: while tranche-0 k-tiles arrive (~0.75us apart), matmuls run
  kt-OUTER across 4 live banks (bt0,bt1)x(nt0,nt1); bt2 joins from
  kt6 (6 live banks) and wraps around to finish kt0..5 afterwards.
  Transposes for all bts fill the remaining PE slack.
- Steady state: per-(bt,nt) single-bank groups, kt-inner, 16 matmuls
  accumulating in one PSUM bank, evicted alternately by ACT/DVE.
- Output DMAs ride the GPSIMD (SWDGE) ring: the SP ring stays clean
  for input streaming and the ACT ring for compute, so no DMA-wait
  ever head-of-line-blocks an engine's compute stream.
- x cast f32->bf16 on ACT in 512-col chunks; PE transposes 4 blocks
  per PSUM staging tile; one contiguous ACT eviction into per-bt xT.
"""

import numpy as np

import concourse.bass as bass
import concourse.mybir as mybir
import concourse.tile as tile
from concourse import bacc
from concourse.bass_utils import run_bass_kernel_spmd
from concourse.masks import make_identity

B, K, N = 8192, 2048, 2048
N_CORES = 8
MB = B // N_CORES          # 1024 batch rows per core
P = 128
KT = K // P                # 16 k-tiles
BT = MB // P               # 8 batch tiles per core
NT = 4                     # output column blocks of 512
NB = N // NT               # 512
HW = K // 2                # 1024

F32 = mybir.dt.float32
BF16 = mybir.dt.bfloat16

WAVE_JOIN_KT = 4           # kt at which bt2 joins the ramp wave


def build_kernel(repeat: int = 1, mode: str = "full"):
    nc = bacc.Bacc(None, target_bir_lowering=False)
    x = nc.dram_tensor("x", [MB, K], F32, kind="ExternalInput")
    w = nc.dram_tensor("w", [K, N], F32, kind="ExternalInput")
    out = nc.dram_tensor("out", [MB, N], F32, kind="ExternalOutput")

    w3 = w[:].rearrange("(kt p) n -> p kt n", p=P)   # [128, 16, 2048]

    do_x = mode in ("full", "nomm", "xonly")
    do_w = mode in ("full", "nomm", "wonly")
    do_mm = mode in ("full", "mmonly")

    def body(tc, pools):
        (xraw_pool, xbf_pool, xT_pool, wraw_pool, wbin_pool,
         out_pool, psum_pool, tpsum_pool, const_pool) = pools

        ident = const_pool.tile([P, P], BF16, tag="ident", name="ident")
        make_identity(nc, ident)

        xraw = {}   # (bt, half) -> [P, HW] f32
        xT = {}     # bt -> [P, K] bf16   (col = kt*P + m)
        wbin = {}   # (kt, nt) -> [P, NB] bf16
        evict_flip = [0]

        def dma_x(bt, half, quarters=False):
            if not do_x:
                return
            t = xraw_pool.tile([P, HW], F32, tag=f"xraw_{half}",
                               name=f"xr{bt}_{half}")
            h0 = half * HW
            if quarters:
                nc.sync.dma_start(t[:, :NB], x[bt * P:(bt + 1) * P,
                                               h0:h0 + NB])
                nc.sync.dma_start(t[:, NB:], x[bt * P:(bt + 1) * P,
                                               h0 + NB:h0 + HW])
            else:
                nc.sync.dma_start(t[:], x[bt * P:(bt + 1) * P, h0:h0 + HW])
            xraw[bt, half] = t

        def dma_w2(kt):
            """Tranche-0: [128,1024] w k-tile -> binarized quarters nt0,nt1."""
            for j in range(2):
                wbin[kt, j] = wbin_pool.tile(
                    [P, NB], BF16, tag=f"wbin_{kt}_{j}", name=f"wb{kt}_{j}")
            if do_w:
                wr = wraw_pool.tile([P, HW], F32, tag="wraw2", name="wr")
                nc.sync.dma_start(wr[:], w3[:, kt, 0:HW])
                for j in range(2):
                    nc.vector.tensor_scalar(
                        out=wbin[kt, j][:], in0=wr[:, j * NB:(j + 1) * NB],
                        scalar1=0.0, scalar2=None,
                        op0=mybir.AluOpType.is_gt)
            else:
                for j in range(2):
                    nc.any.memset(wbin[kt, j][:], 1.0)

        def dma_w1(kt, nt):
            """Tranche-2/3: [128,512] w k-tile quarter."""
            wbin[kt, nt] = wbin_pool.tile(
                [P, NB], BF16, tag=f"wbin_{kt}_{nt}", name=f"wb{kt}_{nt}")
            if do_w:
                wr = wraw_pool.tile([P, NB], F32, tag="wraw1", name="wr")
                nc.sync.dma_start(wr[:], w3[:, kt, nt * NB:(nt + 1) * NB])
                nc.vector.tensor_scalar(
                    out=wbin[kt, nt][:], in0=wr[:],
                    scalar1=0.0, scalar2=None, op0=mybir.AluOpType.is_gt)
            else:
                nc.any.memset(wbin[kt, nt][:], 1.0)

        xbf = {}

        def cast_chunk(bt, ktg):
            """ACT: cast 512 cols of x(bt) f32 -> bf16."""
            if bt not in xT:
                xT[bt] = xT_pool.tile([P, K], BF16, tag=f"xT_{bt}",
                                      name=f"xT_{bt}")
            if not do_x:
                if ktg == 0:
                    nc.any.memset(xT[bt][:], 1.0)
                return
            half, off = divmod(ktg * 4 * P, HW)
            xb = xbf_pool.tile([P, 4 * P], BF16, tag=f"xbf_{ktg % 2}",
                               name=f"xbf{bt}_{ktg}")
            nc.scalar.activation(
                xb[:], xraw[bt, half][:, off:off + 4 * P],
                mybir.ActivationFunctionType.Copy)
            xbf[bt, ktg] = xb

        def transp_chunk(bt, ktg):
            """PE: transpose 4 blocks into one PSUM staging tile; ACT
            evicts contiguously into xT[bt]."""
            if not do_x:
                return
            xb = xbf.pop((bt, ktg))
            tp = tpsum_pool.tile([P, 4 * P], BF16, tag="tps", name="tp")
            for i in range(4):
                nc.tensor.transpose(tp[:, i * P:(i + 1) * P],
                                    xb[:, i * P:(i + 1) * P], ident[:])
            nc.scalar.activation(
                xT[bt][:, ktg * 4 * P:(ktg + 1) * 4 * P], tp[:],
                mybir.ActivationFunctionType.Copy)

        def cast_T(bt, ktg):
            cast_chunk(bt, ktg)
            transp_chunk(bt, ktg)

        def mm(ps, bt, nt, kt, start, stop):
            nc.tensor.matmul(
                ps[:], xT[bt][:, kt * P:(kt + 1) * P], wbin[kt, nt][:],
                start=start, stop=stop)

        def evict_out(ps, bt, nt, split=1, ring=None):
            """PSUM -> SBUF staging (ACT/DVE alternate) -> DRAM via the
            GPSIMD SWDGE ring (keeps SP/ACT rings free). The tail groups
            use the SP HWDGE ring instead: inputs are long done and HWDGE
            start latency beats the ~1us serial SWDGE generation."""
            ring = ring or nc.gpsimd
            c = NB // split
            for s in range(split):
                ot = out_pool.tile([P, c], F32, tag="osb", name="ot")
                evict_flip[0] ^= 1
                if evict_flip[0] == 0:
                    nc.vector.tensor_copy(ot[:], ps[:, s * c:(s + 1) * c])
                else:
                    nc.scalar.activation(ot[:], ps[:, s * c:(s + 1) * c],
                                         mybir.ActivationFunctionType.Copy)
                ring.dma_start(
                    out[bt * P:(bt + 1) * P,
                        nt * NB + s * c:nt * NB + (s + 1) * c], ot[:])

        def group(bt, nt, split_tail=1, ring=None):
            ps = psum_pool.tile([P, NB], F32, tag="ps", name="ps")
            if do_mm:
                for kt in range(KT):
                    mm(ps, bt, nt, kt, kt == 0, kt == KT - 1)
            else:
                nc.any.memset(ps[:], 0.0)
            evict_out(ps, bt, nt, split=split_tail, ring=ring)

        # ================= emission script =================
        # startup: first x chunks and w k-tiles interleaved so the first
        # wave matmul can issue as early as possible
        dma_x(0, 0, quarters=True)
        dma_w2(0)
        dma_x(0, 1)
        dma_w2(1)
        dma_x(1, 0)
        dma_x(1, 1)
        dma_w2(2)
        dma_x(2, 0)
        dma_x(2, 1)
        for kt in range(3, KT):
            dma_w2(kt)
            if kt == 12:
                dma_x(3, 0)
                dma_x(3, 1)
        # rest of x right after tranche-0; then tranches 2 and 3
        for bt in range(4, BT):
            dma_x(bt, 0)
            dma_x(bt, 1)
        for kt in range(KT):
            dma_w1(kt, 2)
        for kt in range(KT):
            dma_w1(kt, 3)

        # transposes for bt0, bt1 up front (PE warms up on these)
        for bt in (0, 1):
            for ktg in range(4):
                cast_T(bt, ktg)

        # ramp wave: kt-outer, (bt0,bt1)x(nt0,nt1); bt2 joins at kt4 and
        # wraps around; transposes for bt2 fill PE slack, bt3's casts are
        # prefetched on ACT near the wave end (its x lands ~kt13)
        if do_mm:
            wave = [(0, 0), (0, 1), (1, 0), (1, 1)]
            pss = {g: psum_pool.tile([P, NB], F32, tag="ps", name=f"wps{g}")
                   for g in wave}
            join = [(2, 0), (2, 1)]
            for kt in range(KT):
                for bt_, nt_ in wave:
                    mm(pss[bt_, nt_], bt_, nt_, kt, kt == 0, kt == KT - 1)
                if kt == WAVE_JOIN_KT - 2:
                    cast_T(2, 0)
                    cast_T(2, 1)
                elif kt == WAVE_JOIN_KT - 1:
                    cast_T(2, 2)
                    cast_T(2, 3)
                    for g in join:
                        pss[g] = psum_pool.tile([P, NB], F32, tag="ps",
                                                name=f"wps{g}")
                if kt >= WAVE_JOIN_KT:
                    for bt_, nt_ in join:
                        mm(pss[bt_, nt_], bt_, nt_, kt,
                           kt == WAVE_JOIN_KT, False)
                if kt == 13:
                    cast_chunk(3, 0)
                    cast_chunk(3, 1)
                elif kt == 15:
                    cast_chunk(3, 2)
                    cast_chunk(3, 3)
            # bt0/bt1 banks are complete: evict them while the PE finishes
            # bt2's wrap-around (kt0..3), freeing banks for group(3, *)
            for g in wave:
                evict_out(pss[g], g[0], g[1])
            for kt in range(WAVE_JOIN_KT):
                for bt_, nt_ in join:
                    mm(pss[bt_, nt_], bt_, nt_, kt, False,
                       kt == WAVE_JOIN_KT - 1)
            for g in (0, 1, 2, 3):
                transp_chunk(3, g)
            for g in join:
                evict_out(pss[g], g[0], g[1])
        else:
            for bt_ in (2, 3):
                for g in range(4):
                    cast_T(bt_, g)

        # steady state: tranche-0 groups for bt3..7; transposes for bt4..7
        # placed just after their x tile has arrived
        tplan = {(3, 0): (4, (0, 1)), (3, 1): (4, (2, 3)),
                 (4, 0): (5, (0, 1)), (4, 1): (5, (2, 3)),
                 (5, 0): (6, (0, 1)), (5, 1): (6, (2, 3)),
                 (6, 0): (7, (0, 1)), (6, 1): (7, (2, 3))}
        for bt in range(3, BT):
            for nt in (0, 1):
                group(bt, nt)
                if (bt, nt) in tplan:
                    b2, ktgs = tplan[bt, nt]
                    for g in ktgs:
                        cast_T(b2, g)

        # tranche-2 then tranche-3 groups for all bts; the last two groups
        # ride the SP HWDGE ring (inputs done; lower latency than SWDGE)
        # and the final eviction is split so the tail out-DMA starts early
        for bt in range(BT):
            group(bt, 2)
        for bt in range(BT):
            last = bt == BT - 1
            group(bt, 3, split_tail=2 if last else 1,
                  ring=nc.sync if bt >= BT - 2 else None)

    with tile.TileContext(nc) as tc:
        with (
            tc.tile_pool(name="xraw", bufs=3) as xraw_pool,
            tc.tile_pool(name="xbf", bufs=2) as xbf_pool,
            tc.tile_pool(name="xT", bufs=1) as xT_pool,
            tc.tile_pool(name="wraw", bufs=4) as wraw_pool,
            tc.tile_pool(name="wbin", bufs=1) as wbin_pool,
            tc.tile_pool(name="osb", bufs=6) as out_pool,
            tc.tile_pool(name="ps", bufs=6, space="PSUM") as psum_pool,
            tc.tile_pool(name="tps", bufs=2, space="PSUM") as tpsum_pool,
            tc.tile_pool(name="const", bufs=1) as const_pool,
        ):
            pools = (xraw_pool, xbf_pool, xT_pool, wraw_pool, wbin_pool,
                     out_pool, psum_pool, tpsum_pool, const_pool)
            if repeat == 1:
                body(tc, pools)
            else:
                with tc.For_i(0, repeat, 1):
                    body(tc, pools)
    _dedup_ldweights(nc)
    nc.compile()
    return nc


def _ldw_key(ins):
    ap = ins.ins[0]
    bap = getattr(ap, "bass_ap", None)
    return (getattr(ap, "memref", None), getattr(bap, "offset", None),
            str(getattr(bap, "ap", None)), getattr(ins, "is_transpose", None))


def _dedup_ldweights(nc):
    """Remove PE weight reloads of the already-loaded stationary operand."""
    removed = 0
    for bb in nc.main_func.blocks:
        il = bb.instructions
        last_key = None
        drop = []
        for idx, ins in enumerate(il):
            if not isinstance(ins, mybir.InstLdweights):
                continue
            si = ins.sync_info
            has_sync = si is not None and (
                (si.on_wait and len(si.on_wait) > 0)
                or (si.on_update and len(si.on_update) > 0))
            key = _ldw_key(ins)
            if key == last_key and not has_sync:
                drop.append(idx)
                removed += 1
            else:
                last_key = key
        for idx in reversed(drop):
            del il[idx]
    return removed


_NC_CACHE = None


def _get_nc():
    global _NC_CACHE
    if _NC_CACHE is None:
        _NC_CACHE = build_kernel()
    return _NC_CACHE


def kernel(x: np.ndarray, weight: np.ndarray):
    assert x.shape == (B, K) and weight.shape == (K, N)
    x = np.ascontiguousarray(x, dtype=np.float32)
    weight = np.ascontiguousarray(weight, dtype=np.float32)
    nc = _get_nc()
    in_maps = [
        {"x": x[i * MB:(i + 1) * MB], "w": weight}
        for i in range(N_CORES)
    ]
    res = run_bass_kernel_spmd(nc, in_maps, core_ids=list(range(N_CORES)))
    return np.concatenate([res.results[i]["out"] for i in range(N_CORES)], axis=0)
